# revision 41
# baseline (speedup 1.0000x reference)
"""Trainium2 Bass kernel for ClusterEncoder (segment mean-pool -> Linear -> gather).

Reference semantics (per batch b):
    sums[c]   = sum_{p: ids[b,p]==c} x[b,p,:]          # [C, E]
    counts[c] = #{p: ids[b,p]==c}
    means     = sums / max(counts, 1)
    Y         = means @ W.T + bias                      # [C, E]
    g_node[p] = Y[ids[b,p]]                             # [P, E]
Outputs: (cluster_embs=[B,C,E], g_node=[B,P,E])

Strategy: pure data parallel over 8 NeuronCores (8 batches each).
Per batch on-device:
  - one-hot H tiles [128,C] built on DVE via tensor_scalar(is_equal) against
    an iota row; node ids (pre-tiled/padded on host) give the scalars.
  - segment sums AND counts in one PE accumulation: psum[C, E+1] += H.T @ [X|1]
    (ones column lives at col E of each persistent X slot, memset once).
  - means via reciprocal(max(counts,1)) * sums; project with two small MMs
    (PE transpose of means, then meansT.T @ W.T); bias add on DVE.
  - gather-back as PE matmul: g_tile = Ht.T @ Y with Ht = PE-transpose of H
    (Ht tiles staged in SBUF between the two passes).

Precision modes (PE fp32 matmul costs 4 cycles/row; bf16 costs 1):
  - "f32":   all-fp32 matmuls (exact, slowest PE)
  - "split": X and Y are split on the host into hi+lo bf16 pairs; each matmul
    becomes two bf16 matmuls accumulating in fp32 PSUM (one-hot weights are
    exact in bf16).  ~2^-16 relative data error, 2x PE win over f32.
  - "bf16":  single bf16 pass (~4e-3 error, fastest)
"""

import sys

sys.path.insert(0, "/opt/trn_rl_repo")

import numpy as np

import concourse.bacc as bacc
import concourse.tile as tile
from concourse import mybir
from concourse.bass_utils import run_bass_kernel_spmd

F32 = mybir.dt.float32
BF16 = mybir.dt.bfloat16

B, P, E, C = 64, 10000, 128, 100
NCORES = 8
BS = B // NCORES   # batches per core
TP = 128           # nodes per tile (partition dim)
XCHUNK = 16        # tiles per X-load DMA
GGRP = 4           # gather tiles per fp32 PSUM bank
HGRP = 8           # Ht tiles per bf16 PSUM bank / copy
OGRP = 8           # gather tiles per output DMA

MODE = "split"     # default precision mode


def _mode_cfg(mode):
    if mode == "f32":
        return F32, 1
    if mode == "split":
        return BF16, 2
    if mode == "bf16":
        return BF16, 1
    raise ValueError(mode)


def build_nc(bs=BS, p=P, num_devices=NCORES, mode=MODE):
    """Build + finalize the per-core Bass program."""
    DT, XH = _mode_cfg(mode)
    nt = (p + TP - 1) // TP            # node tiles per batch
    tail = p - (nt - 1) * TP           # valid rows in last tile
    nch = (nt + XCHUNK - 1) // XCHUNK  # X-load chunks
    nhgrp = (nt + HGRP - 1) // HGRP    # Ht staging groups
    nogrp = (nt + OGRP - 1) // OGRP    # gather output groups

    nc = bacc.Bacc("TRN2", target_bir_lowering=False, debug=False,
                   num_devices=num_devices)

    XW = E + 4  # row width per half: E cols of X, col E = 1.0/0.0, 3 pad
    # xp: host-pretiled, padded X with embedded ones column:
    # [bs, nt, TP, XH, XW] in DT
    xp = nc.dram_tensor("xp", [bs, nt, TP, XH, XW], DT, kind="ExternalInput")
    ids_t = nc.dram_tensor("ids_t", [bs, TP, nt], F32, kind="ExternalInput")
    w_t = nc.dram_tensor("w_t", [E, E], F32, kind="ExternalInput")
    bias_r = nc.dram_tensor("bias_r", [TP, E], F32, kind="ExternalInput")
    iota_r = nc.dram_tensor("iota_r", [TP, C], DT, kind="ExternalInput")
    ident_d = nc.dram_tensor("ident_d", [TP, TP], DT, kind="ExternalInput")
    identf_d = nc.dram_tensor("identf_d", [TP, TP], F32, kind="ExternalInput")
    ce = nc.dram_tensor("ce", [bs, C, E], F32, kind="ExternalOutput")
    gn = nc.dram_tensor("gn", [bs, p, E], F32, kind="ExternalOutput")

    eq = mybir.AluOpType.is_equal

    with tile.TileContext(nc) as tc:
        with (
            tc.tile_pool(name="const", bufs=1) as const,
            tc.tile_pool(name="ids", bufs=2) as idsp,
            tc.tile_pool(name="h", bufs=6) as hp,
            tc.tile_pool(name="mid", bufs=2) as midp,
            tc.tile_pool(name="gsb", bufs=4) as gsbp,
            tc.tile_pool(name="segps", bufs=2, space="PSUM") as segpp,
            tc.tile_pool(name="htps", bufs=2, space="PSUM") as htpp,
            tc.tile_pool(name="gps", bufs=3, space="PSUM") as gpp,
            tc.tile_pool(name="midps", bufs=1, space="PSUM") as midpp,
        ):
            # constants
            iota_sb = const.tile([TP, C], DT, tag="iota")
            w_sb = const.tile([E, E], F32, tag="w")
            bias_sb = const.tile([TP, E], F32, tag="bias")
            ident_sb = const.tile([TP, TP], DT, tag="ident")
            identf_sb = const.tile([TP, TP], F32, tag="identf")
            nc.sync.dma_start(out=iota_sb[:], in_=iota_r[:])
            nc.sync.dma_start(out=w_sb[:], in_=w_t[:])
            nc.sync.dma_start(out=bias_sb[:], in_=bias_r[:])
            nc.sync.dma_start(out=ident_sb[:], in_=ident_d[:])
            nc.sync.dma_start(out=identf_sb[:], in_=identf_d[:])

            # persistent X slots: [TP, XCHUNK, XH, XW]; the host embeds the
            # counts-ones column at [.., 0, E] and zeros at [.., 1, E]
            NXS = 3
            xslots = [const.tile([TP, XCHUNK, XH, XW], DT, tag=f"xs{j}",
                                 name=f"xs{j}") for j in range(NXS)]

            # persistent Ht group tiles, double banked by batch parity
            htb = [[const.tile([TP, HGRP, TP], DT, tag=f"ht{pb}_{g}",
                               name=f"ht{pb}_{g}")
                    for g in range(nhgrp)] for pb in range(2)]
            # Y (projected cluster embeddings) in DT halves, double banked
            yb = [[const.tile([TP, E], DT, tag=f"y{pb}_{h}", name=f"y{pb}_{h}")
                   for h in range(XH)] for pb in range(2)]

            for b in range(bs):
                pb = b % 2
                ids_sb = idsp.tile([TP, nt], F32)
                nc.sync.dma_start(out=ids_sb[:], in_=ids_t[b])

                seg_ps = segpp.tile([C, XH, E + 1], F32)

                # ---- pass 1: segment sums/counts + Ht staging ----
                for ci in range(nch):
                    t0 = ci * XCHUNK
                    ntile = min(XCHUNK, nt - t0)
                    s = xslots[ci % NXS]
                    nc.sync.dma_start(
                        out=s[:, 0:ntile, :, :],
                        in_=xp[b, t0:t0 + ntile].rearrange(
                            "t pp h e -> pp t h e"),
                    )
                    for t in range(ntile):
                        i = t0 + t
                        h = hp.tile([TP, C], DT)
                        nc.vector.tensor_scalar(
                            out=h[:], in0=iota_sb[:],
                            scalar1=ids_sb[:, i:i + 1], scalar2=None, op0=eq)
                        # one MM per tile: both halves side by side in PSUM
                        nc.tensor.matmul(
                            out=seg_ps[:], lhsT=h[:],
                            rhs=s[:, t, :, 0:E + 1],
                            start=(i == 0), stop=(i == nt - 1))
                        g, gt = i // HGRP, i % HGRP
                        if gt == 0:
                            ht_ps = htpp.tile([TP, HGRP, TP], DT)
                        nc.tensor.transpose(
                            out=ht_ps[0:C, gt, :], in_=h[:], identity=ident_sb[:])
                        if gt == HGRP - 1 or i == nt - 1:
                            eng = nc.vector if g % 2 == 0 else nc.scalar
                            dst = htb[pb][g]
                            if eng is nc.vector:
                                eng.tensor_copy(out=dst[0:C, 0:gt + 1, :],
                                                in_=ht_ps[0:C, 0:gt + 1, :])
                            else:
                                eng.copy(out=dst[0:C, 0:gt + 1, :],
                                         in_=ht_ps[0:C, 0:gt + 1, :])

                # ---- mid: means -> projection -> Y (fp32) -> DT halves ----
                cntm = midp.tile([C, 1], F32, tag="cntm")
                nc.vector.tensor_scalar(out=cntm[:], in0=seg_ps[:, 0, E:E + 1],
                                        scalar1=1.0, scalar2=None,
                                        op0=mybir.AluOpType.max)
                rcp = midp.tile([C, 1], F32, tag="rcp")
                nc.vector.reciprocal(out=rcp[:], in_=cntm[:])
                if XH == 2:
                    slo = midp.tile([C, E], F32, tag="slo")
                    nc.vector.tensor_copy(out=slo[:], in_=seg_ps[:, 1, 0:E])
                    sums = midp.tile([C, E], F32, tag="sums")
                    nc.vector.tensor_tensor(out=sums[:], in0=seg_ps[:, 0, 0:E],
                                            in1=slo[:],
                                            op=mybir.AluOpType.add)
                    sums_src = sums[:]
                else:
                    sums_src = seg_ps[:, 0, 0:E]
                means = midp.tile([C, E], F32, tag="means")
                nc.vector.tensor_scalar(out=means[:], in0=sums_src,
                                        scalar1=rcp[:], scalar2=None,
                                        op0=mybir.AluOpType.mult)
                mt_ps = midpp.tile([E, C], F32, tag="mid")
                nc.tensor.transpose(out=mt_ps[:], in_=means[:],
                                    identity=identf_sb[0:C, 0:C])
                mt = midp.tile([E, C], F32, tag="mt")
                nc.vector.tensor_copy(out=mt[:], in_=mt_ps[:])
                y_ps = midpp.tile([C, E], F32, tag="mid")
                nc.tensor.matmul(out=y_ps[:], lhsT=mt[:], rhs=w_sb[:],
                                 start=True, stop=True)
                yf = midp.tile([C, E], F32, tag="yf")
                nc.vector.tensor_tensor(out=yf[:], in0=y_ps[:],
                                        in1=bias_sb[0:C, :],
                                        op=mybir.AluOpType.add)
                nc.scalar.dma_start(out=ce[b], in_=yf[:])
                # split Y into DT halves for the gather
                y0 = yb[pb][0]
                nc.vector.tensor_copy(out=y0[0:C, :], in_=yf[:])
                if XH == 2:
                    y1 = yb[pb][1]
                    nc.vector.tensor_tensor(out=y1[0:C, :], in0=yf[:],
                                            in1=y0[0:C, :],
                                            op=mybir.AluOpType.subtract)

                # ---- pass 2: gather-back ----
                for og in range(nogrp):
                    o0 = og * OGRP
                    ont = min(OGRP, nt - o0)
                    g_sb = gsbp.tile([TP, OGRP, E], F32)
                    for q in range((ont + GGRP - 1) // GGRP):
                        i0 = o0 + q * GGRP
                        gnt = min(GGRP, nt - i0, o0 + ont - i0)
                        g_ps = gpp.tile([TP, GGRP, E], F32)
                        for t in range(gnt):
                            i = i0 + t
                            lhs = htb[pb][i // HGRP][0:C, i % HGRP, :]
                            for hh in range(XH):
                                nc.tensor.matmul(
                                    out=g_ps[:, t, :], lhsT=lhs,
                                    rhs=yb[pb][hh][0:C, :],
                                    start=(hh == 0), stop=(hh == XH - 1))
                        eng = nc.vector if q % 2 == 0 else nc.scalar
                        dst = g_sb[:, q * GGRP:q * GGRP + gnt, :]
                        if eng is nc.vector:
                            eng.tensor_copy(out=dst, in_=g_ps[:, 0:gnt, :])
                        else:
                            eng.copy(out=dst, in_=g_ps[:, 0:gnt, :])
                    # tiles with full 128 valid rows in this output group
                    ofull = ont - 1 if o0 + ont == nt and tail < TP else ont
                    r0 = o0 * TP
                    if ofull > 0:
                        nc.scalar.dma_start(
                            out=gn[b, r0:r0 + ofull * TP, :].rearrange(
                                "(t pp) e -> pp t e", pp=TP),
                            in_=g_sb[:, 0:ofull, :])
                    if ofull < ont:
                        nc.scalar.dma_start(
                            out=gn[b, r0 + ofull * TP:p, :].rearrange(
                                "(t pp) e -> pp t e", pp=tail),
                            in_=g_sb[0:tail, ofull:ofull + 1, :])

    nc.finalize()
    return nc


def _split_hi_lo(x):
    from ml_dtypes import bfloat16
    hi = x.astype(bfloat16)
    lo = (x - hi.astype(np.float32)).astype(bfloat16)
    return hi, lo


def _prep_inputs(encoded_nodes, cluster_ids, proj_w, proj_b, bs=BS, p=P,
                 mode=MODE):
    """Host-side input prep (cast/layout only)."""
    from ml_dtypes import bfloat16
    DT, XH = _mode_cfg(mode)
    np_dt = np.float32 if DT is F32 else bfloat16
    nt = (p + TP - 1) // TP
    XW = E + 4
    x = np.asarray(encoded_nodes, dtype=np.float32)
    nb = x.shape[0]
    # pad rows to nt*TP with zeros, tile to [nb, nt, TP, XH, XW];
    # col E of half 0 = 1.0 (counts ones), col E of half 1 = 0.0
    xpad = np.zeros((nb, nt * TP, E), np.float32)
    xpad[:, :p] = x
    xpad = xpad.reshape(nb, nt, TP, E)
    xp = np.zeros((nb, nt, TP, XH, XW), np_dt)
    if XH == 2:
        hi, lo = _split_hi_lo(xpad)
        xp[:, :, :, 0, :E] = hi
        xp[:, :, :, 1, :E] = lo
    else:
        xp[:, :, :, 0, :E] = xpad.astype(np_dt)
    xp[:, :, :, 0, E] = 1.0
    ids = np.asarray(cluster_ids)
    ids_pad = np.full((nb, nt * TP), C, dtype=np.float32)
    ids_pad[:, :p] = ids.astype(np.float32)
    ids_t = np.ascontiguousarray(ids_pad.reshape(nb, nt, TP).transpose(0, 2, 1))
    w_t = np.ascontiguousarray(np.asarray(proj_w, dtype=np.float32).T)
    bias_r = np.ascontiguousarray(
        np.broadcast_to(np.asarray(proj_b, dtype=np.float32), (TP, E)))
    iota_r = np.ascontiguousarray(
        np.broadcast_to(np.arange(C).astype(np_dt), (TP, C)))
    ident = np.eye(TP, dtype=np_dt)
    per_core = []
    ncores = nb // bs
    for c in range(ncores):
        per_core.append({
            "xp": xp[c * bs:(c + 1) * bs],
            "ids_t": ids_t[c * bs:(c + 1) * bs],
            "w_t": w_t, "bias_r": bias_r, "iota_r": iota_r, "ident_d": ident,
            "identf_d": np.eye(TP, dtype=np.float32),
        })
    return per_core


_NC_CACHE = {}


def _get_nc(bs=BS, p=P, mode=MODE):
    key = (bs, p, mode)
    if key not in _NC_CACHE:
        _NC_CACHE[key] = build_nc(bs, p, mode=mode)
    return _NC_CACHE[key]


def kernel(encoded_nodes, cluster_ids, num_clusters, proj_w, proj_b):
    assert int(num_clusters) == C
    nc = _get_nc()
    in_maps = _prep_inputs(encoded_nodes, cluster_ids, proj_w, proj_b)
    res = run_bass_kernel_spmd(nc, in_maps, core_ids=list(range(NCORES)))
    cluster_embs = np.concatenate([r["ce"] for r in res.results], axis=0)
    g_node = np.concatenate([r["gn"] for r in res.results], axis=0)
    return cluster_embs, g_node
